# revision 2
# baseline (speedup 1.0000x reference)
"""DiscMaker scan kernel for TRN2 (8 NeuronCores, pure data-parallel over batch).

Layout: feature-on-partition, batch-on-free ("transposed"). All per-step
matmuls are out^T[M,B] = lhsT.T @ rhs with lhsT = weight [K, M], rhs =
activation [K, B].  The S axis is rolled by 1 so pred = kstate row 0.
Softmax over the M=8 models is done with matmuls (colsum via ones, broadcast
via ones, expand via 0/1 selector matrices).
"""

import numpy as np
from contextlib import ExitStack

import concourse.bass as bass
import concourse.tile as tile
from concourse import mybir
from concourse.bass_utils import run_bass_kernel_spmd

AF = mybir.ActivationFunctionType
OP = mybir.AluOpType
f32 = mybir.dt.float32

B_TOT, IN, S, M, E, H = 128, 32, 32, 8, 64, 128
NCORES = 8
Bc = B_TOT // NCORES  # 16
TC = 128              # timesteps per DMA chunk

# name -> (partitions, cols) for the packed weight buffer (one DMA)
W_SHAPES = {
    "Wenc": (IN, E), "benc": (E, 1),
    "Win0": (IN, 128), "Win1": (IN, 128),
    "Wrec0": (S, 128), "Wrec1": (S, 128),
    "bs0": (128, 1), "bs1": (128, 1),
    "Wxe": (E, 3 * H), "Wxr": (1, 3 * H), "Wh": (H, 3 * H),
    "bz": (H, 1), "br": (H, 1), "bxn": (H, 1), "bhn": (H, 1),
    "Wg": (H, M), "bg": (M, 1),
    "ones8": (M, 1), "ones1_8": (1, M),
    "Sel0": (M, 128), "Sel1": (M, 128),
    "R0": (128, S), "R1": (128, S),
}


def _build(T: int) -> bass.Bass:
    """Raw-bass build: fully serial global order via one semaphore.

    This toolchain's walrus rejects >1 sync wait attached to an instruction,
    so all cross-engine waits are emitted as standalone wait_ge ops and every
    instruction increments one global semaphore (DMA: +16, compute: +1)."""
    nc = bass.Bass()

    def dparam(name, shp, out=False):
        return nc.declare_dram_parameter(name, list(shp), f32, isOutput=out)

    xr = dparam("xr", (IN, T, Bc))
    yr = dparam("yr", (1, T, Bc))
    ncol = sum(c for _, c in W_SHAPES.values())
    wpack_d = dparam("wpack", (128, ncol))
    preds = dparam("preds", (1, T, Bc), out=True)

    ops = []  # (engine, emit_fn, inc)

    def op(eng, fn, inc=1):
        ops.append((eng, fn, inc))

    with ExitStack() as ctx:
        sb = lambda shp, name: ctx.enter_context(
            nc.sbuf_tensor(name, list(shp), f32))
        ps = lambda shp, name: ctx.enter_context(
            nc.psum_tensor(name, list(shp), f32))

        wtile = sb((128, ncol), "wtile")
        W = {}
        off = 0
        for k, (p, c) in W_SHAPES.items():
            W[k] = wtile[0:p, off:off + c]
            off += c
        xch = sb((IN, TC, Bc), "xch")
        ych = sb((1, TC, Bc), "ych")
        pch = sb((1, TC, Bc), "pch")
        ks = sb((S, Bc), "ks")
        h = sb((H, Bc), "h")
        err = sb((1, Bc), "err")
        enc = sb((E, Bc), "enc")
        ns0 = sb((128, Bc), "ns0")
        ns1 = sb((128, Bc), "ns1")
        z = sb((H, Bc), "z")
        r = sb((H, Bc), "rr")
        rhn = sb((H, Bc), "rhn")
        t3 = sb((H, Bc), "t3")
        nn_ = sb((H, Bc), "nn")
        hmn = sb((H, Bc), "hmn")
        zm = sb((H, Bc), "zm")
        e = sb((M, Bc), "e")
        gate = sb((M, Bc), "gate")
        recip = sb((1, Bc), "recip")
        prod0 = sb((128, Bc), "prod0")
        prod1 = sb((128, Bc), "prod1")

        p_enc = ps((E, Bc), "p_enc")
        p_ns0 = ps((128, Bc), "p_ns0")
        p_ns1 = ps((128, Bc), "p_ns1")
        p_zr = ps((H, 2 * Bc), "p_zr")
        p_n = ps((H, 2 * Bc), "p_n")
        p_sm = ps((M, 48), "p_sm")
        p_eg = ps((128, 2 * Bc), "p_eg")
        p_mix = ps((S, Bc), "p_mix")

        # ---- program (global serial order) ----
        op("sync", lambda: nc.sync.dma_start(out=wtile[:], in_=wpack_d[:]), 16)
        op("vector", lambda: nc.vector.memset(ks[:], 0.0))
        op("vector", lambda: nc.vector.memset(h[:], 0.0))
        op("vector", lambda: nc.vector.memset(err[:], 0.0))

        MM = nc.tensor.matmul
        ACT = nc.scalar.activation
        for c in range(T // TC):
            c0 = c * TC
            op("sync", lambda c0=c0: nc.sync.dma_start(
                out=xch[:], in_=xr[:, c0:c0 + TC, :]), 16)
            op("sync", lambda c0=c0: nc.sync.dma_start(
                out=ych[:], in_=yr[:, c0:c0 + TC, :]), 16)
            for tl in range(TC):
                xt = xch[:, tl, :]
                yt = ych[:, tl, :]
                op("tensor", lambda xt=xt: MM(p_enc[:], W["Wenc"], xt, start=True, stop=True))
                op("scalar", lambda: ACT(enc[:], p_enc[:], AF.Tanh, bias=W["benc"]))
                op("tensor", lambda xt=xt: MM(p_ns0[:], W["Win0"], xt, start=True, stop=False))
                op("tensor", lambda: MM(p_ns0[:], W["Wrec0"], ks[:], start=False, stop=True))
                op("scalar", lambda: ACT(ns0[:], p_ns0[:], AF.Tanh, bias=W["bs0"]))
                op("tensor", lambda xt=xt: MM(p_ns1[:], W["Win1"], xt, start=True, stop=False))
                op("tensor", lambda: MM(p_ns1[:], W["Wrec1"], ks[:], start=False, stop=True))
                op("scalar", lambda: ACT(ns1[:], p_ns1[:], AF.Tanh, bias=W["bs1"]))
                for gi in range(2):
                    sl = p_zr[:, gi * Bc:(gi + 1) * Bc]
                    cs = slice(gi * H, (gi + 1) * H)
                    op("tensor", lambda sl=sl, cs=cs: MM(sl, W["Wh"][:, cs], h[:], start=True, stop=False))
                    op("tensor", lambda sl=sl, cs=cs: MM(sl, W["Wxe"][:, cs], enc[:], start=False, stop=False))
                    op("tensor", lambda sl=sl, cs=cs: MM(sl, W["Wxr"][:, cs], err[:], start=False, stop=True))
                op("scalar", lambda: ACT(z[:], p_zr[:, 0:Bc], AF.Sigmoid, bias=W["bz"]))
                op("scalar", lambda: ACT(r[:], p_zr[:, Bc:2 * Bc], AF.Sigmoid, bias=W["br"]))
                cs = slice(2 * H, 3 * H)
                op("tensor", lambda cs=cs: MM(p_n[:, 0:Bc], W["Wxe"][:, cs], enc[:], start=True, stop=False))
                op("tensor", lambda cs=cs: MM(p_n[:, 0:Bc], W["Wxr"][:, cs], err[:], start=False, stop=True))
                op("tensor", lambda cs=cs: MM(p_n[:, Bc:2 * Bc], W["Wh"][:, cs], h[:], start=True, stop=True))
                op("vector", lambda: nc.vector.scalar_tensor_tensor(
                    rhn[:], p_n[:, Bc:2 * Bc], W["bhn"], r[:], op0=OP.add, op1=OP.mult))
                op("vector", lambda: nc.vector.tensor_add(t3[:], rhn[:], p_n[:, 0:Bc]))
                op("scalar", lambda: ACT(nn_[:], t3[:], AF.Tanh, bias=W["bxn"]))
                op("vector", lambda: nc.vector.tensor_sub(hmn[:], h[:], nn_[:]))
                op("vector", lambda: nc.vector.tensor_mul(zm[:], z[:], hmn[:]))
                op("vector", lambda: nc.vector.tensor_add(h[:], nn_[:], zm[:]))
                op("tensor", lambda: MM(p_sm[0:M, 0:Bc], W["Wg"], h[:], start=True, stop=True))
                op("scalar", lambda: ACT(e[:], p_sm[0:M, 0:Bc], AF.Exp, bias=W["bg"]))
                op("tensor", lambda: MM(p_sm[0:1, 16:16 + Bc], W["ones8"], e[:], start=True, stop=True))
                op("vector", lambda: nc.vector.reciprocal(recip[:], p_sm[0:1, 16:16 + Bc]))
                op("tensor", lambda: MM(p_sm[0:M, 32:32 + Bc], W["ones1_8"], recip[:], start=True, stop=True))
                op("vector", lambda: nc.vector.tensor_mul(gate[:], e[:], p_sm[0:M, 32:32 + Bc]))
                op("tensor", lambda: MM(p_eg[:, 0:Bc], W["Sel0"], gate[:], start=True, stop=True))
                op("tensor", lambda: MM(p_eg[:, Bc:2 * Bc], W["Sel1"], gate[:], start=True, stop=True))
                op("vector", lambda: nc.vector.tensor_mul(prod0[:], p_eg[:, 0:Bc], ns0[:]))
                op("vector", lambda: nc.vector.tensor_mul(prod1[:], p_eg[:, Bc:2 * Bc], ns1[:]))
                op("tensor", lambda: MM(p_mix[:], W["R0"], prod0[:], start=True, stop=False))
                op("tensor", lambda: MM(p_mix[:], W["R1"], prod1[:], start=False, stop=True))
                op("vector", lambda: nc.vector.tensor_copy(ks[:], p_mix[:]))
                op("scalar", lambda tl=tl: nc.scalar.copy(pch[:, tl, :], ks[0:1, :]))
                op("vector", lambda yt=yt: nc.vector.tensor_sub(err[:], ks[0:1, :], yt))
            op("sync", lambda c0=c0: nc.sync.dma_start(
                out=preds[:, c0:c0 + TC, :], in_=pch[:]), 16)

        # ---- emit per-engine bodies with standalone waits ----
        pre = []
        acc = 0
        for eng, fn, inc in ops:
            pre.append(acc)
            acc += inc
        last_eng_before = {}
        prev_eng = None
        # for skipping redundant waits: op i needs wait only if any op since
        # this engine's previous op belongs to a different engine
        gs = ctx.enter_context(nc.semaphore("gs"))
        blk = ctx.enter_context(nc.Block())

        def make_body(eng_name):
            def body(engine):
                prev = None  # global index of this engine's previous op
                for i, (eng, fn, inc) in enumerate(ops):
                    if eng != eng_name:
                        continue
                    engine.wait_ge(gs, pre[i])
                    fn().then_inc(gs, inc)
                    prev = i
            return body

        blk.sync(make_body("sync"))
        blk.tensor(make_body("tensor"))
        blk.scalar(make_body("scalar"))
        blk.vector(make_body("vector"))

    return nc

def _prep_weights(W_enc, b_enc, W_in, W_rec, b_s, W_x, W_h, b_x, b_h,
                  W_gate, b_gate):
    perm = np.roll(np.arange(S), 1)        # row 0 <- original s=31 (pred)
    W_in_p = W_in[:, :, perm]              # [M, IN, S']
    W_rec_p = W_rec[:, perm][:, :, perm]   # [M, S'in, S'out]
    b_s_p = b_s[:, perm]                   # [M, S']
    Win_f = np.ascontiguousarray(W_in_p.transpose(1, 0, 2).reshape(IN, M * S))
    Wrec_f = np.ascontiguousarray(W_rec_p.transpose(1, 0, 2).reshape(S, M * S))
    bs_f = b_s_p.reshape(M * S, 1)
    Sel = np.kron(np.eye(M, dtype=np.float32), np.ones((1, S), np.float32))
    R = np.kron(np.ones((M, 1), np.float32), np.eye(S, dtype=np.float32))
    f = np.float32
    return {
        "Wenc": W_enc, "benc": b_enc.reshape(E, 1),
        "Win0": Win_f[:, :128], "Win1": Win_f[:, 128:],
        "Wrec0": Wrec_f[:, :128], "Wrec1": Wrec_f[:, 128:],
        "bs0": bs_f[:128], "bs1": bs_f[128:],
        "Wxe": W_x[:E], "Wxr": W_x[E:E + 1], "Wh": W_h,
        "bz": (b_x[:H] + b_h[:H]).reshape(H, 1),
        "br": (b_x[H:2 * H] + b_h[H:2 * H]).reshape(H, 1),
        "bxn": b_x[2 * H:].reshape(H, 1), "bhn": b_h[2 * H:].reshape(H, 1),
        "Wg": W_gate, "bg": b_gate.reshape(M, 1),
        "ones8": np.ones((M, 1), f), "ones1_8": np.ones((1, M), f),
        "Sel0": Sel[:, :128], "Sel1": Sel[:, 128:],
        "R0": R[:128], "R1": R[128:],
    }


def kernel(x, y, W_enc, b_enc, W_in, W_rec, b_s, W_x, W_h, b_x, b_h,
           W_gate, b_gate):
    arrs = [np.ascontiguousarray(np.asarray(a, np.float32)) for a in
            (x, y, W_enc, b_enc, W_in, W_rec, b_s, W_x, W_h, b_x, b_h,
             W_gate, b_gate)]
    x, y = arrs[0], arrs[1]
    T = x.shape[1]
    wmap = _prep_weights(*arrs[2:])
    ncol = sum(c for _, c in W_SHAPES.values())
    wpack = np.zeros((128, ncol), np.float32)
    off = 0
    for k, (p, c) in W_SHAPES.items():
        wpack[0:p, off:off + c] = wmap[k]
        off += c

    in_maps = []
    for c in range(NCORES):
        xs = x[c * Bc:(c + 1) * Bc]                    # [Bc, T, IN]
        ys = y[c * Bc:(c + 1) * Bc]                    # [Bc, T]
        m = {"wpack": wpack}
        m["xr"] = np.ascontiguousarray(xs.transpose(2, 1, 0))      # [IN, T, Bc]
        m["yr"] = np.ascontiguousarray(ys.T.reshape(1, T, Bc))
        in_maps.append(m)

    nc = _build(T)
    res = run_bass_kernel_spmd(nc, in_maps, list(range(NCORES)))
    global LAST_RESULTS
    LAST_RESULTS = res

    out = np.empty((B_TOT, T), np.float32)
    for c in range(NCORES):
        p = res.results[c]["preds"].reshape(T, Bc)
        out[c * Bc:(c + 1) * Bc] = p.T
    return out

